# revision 11
# baseline (speedup 1.0000x reference)
"""AttentionPooling1D Trainium2 kernel.

Reference computation (per batch element b):
    scores[s] = x[b, s, :] @ w[0]                  # [S]
    scores    = where(mask[b] != 0, scores, -inf)
    probs     = softmax(scores)                    # [S]
    out[b, :] = probs @ x[b, :, :]                 # [D]

Strategy (memory-bound, one pass over the rows that matter):
  - Shard batch B=64 across 8 cores (8 per core), no communication.
  - The random 0/1 mask keeps only ~50% of rows; masked rows contribute
    exactly 0 to both the softmax numerator and denominator, so they are
    never loaded: per batch the host emits the list of kept row indices
    (padded to a fixed cap with duplicates of row 0) and the kernel
    gathers just those rows with SWDGE dma_gather, rotating the gather
    calls across all 4 SWDGE queues. This halves HBM traffic vs dense.
  - Per gathered chunk [128 rows, D]: VectorE multiplies by the broadcast
    w; ScalarE activation(Copy, accum_out) reduces along the free dim to
    per-row dot products (scores). A per-chunk additive bias (0 for real
    rows, -30000 for padding) makes exp underflow to exactly 0 on padding,
    so padded positions drop out of the softmax; one batched exp per call.
  - TensorE accumulates numerator acc[1, D] += e^T @ x_chunk and the
    denominator l += e^T @ ones in PSUM across all chunks of a batch
    (float32r: fp32 in/out, FP22 multiply, fp32 accumulate, 1 cycle/row).
  - Final: out[b] = acc * (1/l) via VectorE, DMA to DRAM.
  - Fallback: if any batch keeps more rows than the cap (cannot happen for
    p=0.5 masks at S=4096, cap=2304 is ~8 sigma), a dense variant streams
    all of x over both HWDGE queues in 0.5 MB chunks.
"""

import numpy as np

B, S, D = 64, 4096, 1024
N_CORES = 8
B_PC = B // N_CORES      # batches per core
P = 128                  # SBUF partitions
NEG_BIAS = -30000.0      # exp(x + NEG_BIAS) == 0.0 in fp32 for any plausible x

CAP_CHUNKS = 17          # gather capacity per batch, in 128-row chunks
CPC = 4                  # max chunks per dma_gather call
N_SWQ = 4                # SWDGE queues to rotate gather calls across


def _splits(cap, cpc):
    """Chunk counts per dma_gather call covering `cap` chunks, as even as
    possible with each part <= cpc."""
    n_calls = -(-cap // cpc)
    base, rem = divmod(cap, n_calls)
    return [base + 1] * rem + [base] * (n_calls - rem)


def build_gather(b_pc=B_PC, s=S, d=D, cap_chunks=CAP_CHUNKS, cpc=CPC,
                 n_swq=N_SWQ, x_bufs=8, compute="full", use_mm=True):
    import concourse.bacc as bacc
    import concourse.tile as tile
    from concourse import mybir

    f32 = mybir.dt.float32
    f32r = mybir.dt.float32r
    i16 = mybir.dt.int16
    n_half = d // 2
    assert n_half <= 512
    splits = _splits(cap_chunks, cpc)
    calls_pb = len(splits)
    icols_tot = cap_chunks * P // 16     # int16 idx columns per batch

    nc = bacc.Bacc(trn_type="TRN2", target_bir_lowering=False, debug=False,
                   num_swdge_queues=n_swq)
    x_d = nc.declare_dram_parameter("x", [b_pc, s, d], f32, isOutput=False)
    w_d = nc.declare_dram_parameter("w_rep", [P, d], f32, isOutput=False)
    bias_d = nc.declare_dram_parameter("bias", [P, b_pc * cap_chunks], f32,
                                       isOutput=False)
    idx_d = nc.declare_dram_parameter("idx", [P, b_pc * icols_tot], i16,
                                      isOutput=False)
    ones_d = nc.declare_dram_parameter("ones", [P, 2], f32, isOutput=False)
    out_d = nc.declare_dram_parameter("out", [b_pc, d], f32, isOutput=True)

    def mm(ap):
        return ap.bitcast(f32r) if use_mm else ap

    with tile.TileContext(nc) as tc:
        with (
            tc.tile_pool(name="xpool", bufs=x_bufs) as xpool,
            tc.tile_pool(name="ypool", bufs=3) as ypool,
            tc.tile_pool(name="consts", bufs=1) as consts,
            tc.tile_pool(name="small", bufs=8) as small,
            tc.tile_pool(name="outp", bufs=2) as outp,
            tc.tile_pool(name="psum", bufs=2, space="PSUM") as psum_pool,
        ):
            w_sb = consts.tile([P, d], f32)
            nc.sync.dma_start(out=w_sb, in_=w_d[:])
            bias_sb = consts.tile([P, b_pc * cap_chunks], f32)
            nc.sync.dma_start(out=bias_sb, in_=bias_d[:])
            idx_sb = consts.tile([P, b_pc * icols_tot], i16)
            nc.sync.dma_start(out=idx_sb, in_=idx_d[:])
            ones_sb = consts.tile([P, 2], f32)
            nc.sync.dma_start(out=mm(ones_sb), in_=mm(ones_d[:]))

            qi = 0
            for b in range(b_pc):
                acc0 = psum_pool.tile([1, n_half], f32, tag="acc0")
                acc1 = psum_pool.tile([1, n_half], f32, tag="acc1")
                lps = psum_pool.tile([1, 2], f32, tag="l")
                c0 = 0   # first chunk index of this call within the batch
                for h, cnt in enumerate(splits):
                    nidx = cnt * P
                    icols = nidx // 16
                    ic0 = b * icols_tot + (c0 * P) // 16
                    xt = xpool.tile([P, cpc, d], f32, tag="xt")
                    nc.gpsimd.dma_gather(
                        out_ap=mm(xt[:, :cnt, :]),
                        in_ap=mm(x_d[b]),
                        idxs_ap=idx_sb[:, ic0: ic0 + icols],
                        num_idxs=nidx,
                        num_idxs_reg=nidx,
                        elem_size=d,
                        queue_num=qi % n_swq,
                    )
                    qi += 1
                    if compute == "none":
                        c0 += cnt
                        continue
                    scores = small.tile([P, cpc], f32, tag="scores")
                    for j in range(cnt):
                        y = ypool.tile([P, d], f32, tag="y")
                        nc.vector.tensor_mul(y, xt[:, j, :], w_sb)
                        if compute in ("dve2", "full2"):
                            nc.vector.tensor_reduce(
                                scores[:, j: j + 1], y,
                                mybir.AxisListType.X, mybir.AluOpType.add,
                            )
                        elif compute != "dve":
                            nc.scalar.activation(
                                y, y, mybir.ActivationFunctionType.Copy,
                                accum_out=scores[:, j: j + 1],
                            )
                    if compute in ("dve", "dve2"):
                        c0 += cnt
                        continue
                    col0 = b * cap_chunks + c0
                    nc.vector.tensor_add(
                        scores[:, :cnt], scores[:, :cnt],
                        bias_sb[:, col0: col0 + cnt]
                    )
                    e = small.tile([P, cpc], f32, tag="e")
                    er = mm(e)
                    nc.scalar.activation(
                        er[:, :cnt], scores[:, :cnt],
                        mybir.ActivationFunctionType.Exp
                    )
                    for j in range(cnt):
                        c = c0 + j
                        first = c == 0
                        last = c == cap_chunks - 1
                        ej = er[:, j: j + 1]
                        nc.tensor.matmul(acc0, ej, mm(xt[:, j, :n_half]),
                                         start=first, stop=last)
                        nc.tensor.matmul(acc1, ej, mm(xt[:, j, n_half:]),
                                         start=first, stop=last)
                        nc.tensor.matmul(lps, ej, mm(ones_sb),
                                         start=first, stop=last)
                    c0 += cnt
                if compute in ("none", "dve", "dve2"):
                    nc.sync.dma_start(out=out_d[b: b + 1, :],
                                      in_=xt[0:1, 0, :])
                    continue
                linv = small.tile([1, 1], f32, tag="linv")
                nc.vector.reciprocal(linv, lps[:, 0:1])
                ob = outp.tile([1, d], f32, tag="ob")
                nc.vector.tensor_scalar_mul(ob[:, :n_half], acc0, linv)
                nc.vector.tensor_scalar_mul(ob[:, n_half:], acc1, linv)
                nc.sync.dma_start(out=out_d[b: b + 1, :], in_=ob)
    nc.compile()
    return nc


def make_in_maps_gather(x, padding_mask, w, b_pc=B_PC, s=S, d=D,
                        n_cores=N_CORES, cap_chunks=CAP_CHUNKS, cpc=CPC):
    """Host prep for the gather kernel. Returns None if any batch keeps more
    than cap_chunks*128 rows (caller falls back to dense)."""
    x = np.asarray(x, dtype=np.float32)
    padding_mask = np.asarray(padding_mask)
    w = np.asarray(w, dtype=np.float32)
    cap = cap_chunks * P
    icols_tot = cap * 1 // 16
    w_rep = np.ascontiguousarray(np.broadcast_to(w.reshape(1, d), (P, d)))
    keep_counts = (np.asarray(padding_mask) != 0).sum(axis=1)
    if keep_counts.max() > cap:
        return None
    in_maps = []
    for core in range(n_cores):
        xc = np.ascontiguousarray(x[core * b_pc: (core + 1) * b_pc])
        mc = padding_mask[core * b_pc: (core + 1) * b_pc]
        bias_cols = np.zeros((P, b_pc * cap_chunks), dtype=np.float32)
        idx_cols = np.zeros((16, b_pc * icols_tot), dtype=np.int16)
        for b in range(b_pc):
            keep = np.where(mc[b] != 0)[0]
            nk = len(keep)
            idxs = np.zeros(cap, dtype=np.int16)   # pad = dup of row 0
            idxs[:nk] = keep.astype(np.int16)
            biasvec = np.zeros(cap, dtype=np.float32)
            biasvec[nk:] = NEG_BIAS
            bias_cols[:, b * cap_chunks: (b + 1) * cap_chunks] = (
                biasvec.reshape(cap_chunks, P).T
            )
            # idx stream for the whole batch: k -> partition k%16, col k//16
            idx_cols[:, b * icols_tot: (b + 1) * icols_tot] = (
                idxs.reshape(icols_tot, 16).T
            )
        idx_full = np.ascontiguousarray(np.tile(idx_cols, (8, 1)))
        ones = np.ones((P, 2), dtype=np.float32)
        in_maps.append({
            "x": xc, "w_rep": w_rep,
            "bias": np.ascontiguousarray(bias_cols),
            "idx": idx_full, "ones": ones,
        })
    return in_maps


def build_dense(b_pc=B_PC, s=S, d=D, group=4, x_bufs=16, n_queues=2,
                scores_on="dve"):
    """Dense kernel: stream all of x in 0.5 MB [128, d] chunks, alternating
    the two HWDGE queues (sync + scalar). Per chunk, DVE computes y = x * w
    and the row-sum goes to ScalarE (activation Copy + accum_out) on even
    chunks and to DVE tensor_reduce on odd chunks so neither engine becomes
    the bottleneck. exp runs once per `group` chunks on the batched scores
    (with the mask bias added by DVE), then TensorE accumulates numerator
    and denominator into PSUM."""
    import concourse.bacc as bacc
    import concourse.tile as tile
    from concourse import mybir

    cpb = s // P             # chunks per batch
    gpb = cpb // group       # groups per batch
    assert gpb * group == cpb and cpb * P == s

    f32 = mybir.dt.float32
    f32r = mybir.dt.float32r

    nc = bacc.Bacc(trn_type="TRN2", target_bir_lowering=False, debug=False)
    x_d = nc.declare_dram_parameter("x", [b_pc, s, d], f32, isOutput=False)
    w_d = nc.declare_dram_parameter("w_rep", [P, d], f32, isOutput=False)
    bias_d = nc.declare_dram_parameter("bias", [P, b_pc * cpb], f32,
                                       isOutput=False)
    ones_d = nc.declare_dram_parameter("ones", [P, 2], f32, isOutput=False)
    out_d = nc.declare_dram_parameter("out", [b_pc, d], f32, isOutput=True)

    def mm(ap):
        return ap.bitcast(f32r)

    n_half = d // 2
    assert n_half <= 512

    with tile.TileContext(nc) as tc:
        with (
            tc.tile_pool(name="xpool", bufs=x_bufs) as xpool,
            tc.tile_pool(name="ypool", bufs=6) as ypool,
            tc.tile_pool(name="consts", bufs=1) as consts,
            tc.tile_pool(name="small", bufs=8) as small,
            tc.tile_pool(name="outp", bufs=2) as outp,
            tc.tile_pool(name="psum", bufs=2, space="PSUM") as psum_pool,
        ):
            w_sb = consts.tile([P, d], f32)
            nc.sync.dma_start(out=w_sb, in_=w_d[:])
            bias_sb = consts.tile([P, b_pc * cpb], f32)
            nc.sync.dma_start(out=bias_sb, in_=bias_d[:])
            ones_sb = consts.tile([P, 2], f32)
            nc.sync.dma_start(out=mm(ones_sb), in_=mm(ones_d[:]))

            qi = 0
            for b in range(b_pc):
                acc0 = psum_pool.tile([1, n_half], f32, tag="acc0")
                acc1 = psum_pool.tile([1, n_half], f32, tag="acc1")
                lps = psum_pool.tile([1, 2], f32, tag="l")
                for g in range(gpb):
                    xts = []
                    scores = small.tile([P, group], f32, tag="scores")
                    for j in range(group):
                        c = g * group + j
                        xt = xpool.tile([P, d], f32, tag="xt")
                        src = x_d[b, c * P: (c + 1) * P, :]
                        eng = nc.sync if (qi % n_queues) == 0 else nc.scalar
                        eng.dma_start(out=mm(xt), in_=mm(src))
                        qi += 1
                        xts.append(xt)
                        y = ypool.tile([P, d], f32, tag="y")
                        nc.vector.tensor_mul(y, xt, w_sb)
                        use_dve = (scores_on == "dve"
                                   or (scores_on == "split" and j % 2 == 1))
                        if use_dve:
                            nc.vector.tensor_reduce(
                                scores[:, j: j + 1], y,
                                mybir.AxisListType.X, mybir.AluOpType.add,
                            )
                        else:
                            nc.scalar.activation(
                                y, y, mybir.ActivationFunctionType.Copy,
                                accum_out=scores[:, j: j + 1],
                            )
                    col0 = b * cpb + g * group
                    nc.vector.tensor_add(
                        scores, scores, bias_sb[:, col0: col0 + group]
                    )
                    e = small.tile([P, group], f32, tag="e")
                    er = mm(e)
                    nc.scalar.activation(
                        er, scores, mybir.ActivationFunctionType.Exp
                    )
                    for j in range(group):
                        c = g * group + j
                        first = c == 0
                        last = c == cpb - 1
                        ej = er[:, j: j + 1]
                        nc.tensor.matmul(acc0, ej, mm(xts[j][:, :n_half]),
                                         start=first, stop=last)
                        nc.tensor.matmul(acc1, ej, mm(xts[j][:, n_half:]),
                                         start=first, stop=last)
                        nc.tensor.matmul(lps, ej, mm(ones_sb),
                                         start=first, stop=last)
                linv = small.tile([1, 1], f32, tag="linv")
                nc.vector.reciprocal(linv, lps[:, 0:1])
                ob = outp.tile([1, d], f32, tag="ob")
                nc.scalar.activation(ob[:, :n_half], acc0,
                                     mybir.ActivationFunctionType.Copy,
                                     scale=linv)
                nc.scalar.activation(ob[:, n_half:], acc1,
                                     mybir.ActivationFunctionType.Copy,
                                     scale=linv)
                nc.sync.dma_start(out=out_d[b: b + 1, :], in_=ob)
    nc.compile()
    return nc


def make_in_maps_dense(x, padding_mask, w, b_pc=B_PC, s=S, d=D,
                       n_cores=N_CORES):
    x = np.asarray(x, dtype=np.float32)
    padding_mask = np.asarray(padding_mask)
    w = np.asarray(w, dtype=np.float32)
    cpb = s // P
    bias = np.where(padding_mask != 0, np.float32(0.0), np.float32(NEG_BIAS))
    bias = bias.astype(np.float32)
    w_rep = np.ascontiguousarray(np.broadcast_to(w.reshape(1, d), (P, d)))
    in_maps = []
    for core in range(n_cores):
        xc = np.ascontiguousarray(x[core * b_pc: (core + 1) * b_pc])
        bc = bias[core * b_pc: (core + 1) * b_pc]  # [b_pc, s]
        bc = np.ascontiguousarray(
            bc.reshape(b_pc, cpb, P).transpose(2, 0, 1).reshape(P, b_pc * cpb)
        )
        ones = np.ones((P, 2), dtype=np.float32)
        in_maps.append({"x": xc, "w_rep": w_rep, "bias": bc, "ones": ones})
    return in_maps


_NC_CACHE = {}


def _get_nc_gather():
    if "g" not in _NC_CACHE:
        _NC_CACHE["g"] = build_gather()
    return _NC_CACHE["g"]


def _get_nc_dense():
    if "d" not in _NC_CACHE:
        _NC_CACHE["d"] = build_dense()
    return _NC_CACHE["d"]


def kernel(x, padding_mask, w):
    from concourse.bass_utils import run_bass_kernel_spmd

    nc = _get_nc_dense()
    in_maps = make_in_maps_dense(x, padding_mask, w)
    res = run_bass_kernel_spmd(nc, in_maps, list(range(N_CORES)))
    outs = [res.results[c]["out"] for c in range(N_CORES)]
    return np.concatenate(outs, axis=0).astype(np.float32)
